# revision 12
# baseline (speedup 1.0000x reference)
"""BotRGCN forward on 8 Trainium2 NeuronCores (Bass/Tile).

Design: everything feature-major ("transposed") so every matmul is native
lhsT.T @ rhs. Weighted segment-sum over edges = one-hot matmul into PSUM:
S[e, dst] = (iota==dstrel[e]) * ew[e] built on DVE, then
agg_T[feat, dst] += F_edges[e, feat].T @ S. Edge src rows are bulk-gathered
with gpsimd dma_gather (int16 indices within 32k-row windows of the
all-gathered bf16 feature tables). Root weights of the 8 relations sharing
one destination collapse to one host-summed matrix; biases likewise.
"""
import sys, os

sys.path.insert(0, "/opt/trn_rl_repo")
import numpy as np
import ml_dtypes

bfloat16 = ml_dtypes.bfloat16

import concourse.bass as bass
import concourse.bacc as bacc
import concourse.mybir as mybir
import concourse.tile as tile
from concourse.bass_utils import run_bass_kernel_spmd
from concourse import library_config

# ---- wait-legalization (this walrus accepts 1 sync wait per instruction) ----
def legalize_waits(nc):
    for fn in nc.m.functions:
        for bb in fn.blocks:
            insts = list(bb.instructions)
            out = []
            changed = False
            for ins in insts:
                si = ins.sync_info
                waits = list(si.on_wait) if si is not None and si.on_wait else []
                if len(waits) > 1:
                    for w in waits[1:]:
                        nop = mybir.InstNoOp(
                            name=nc.get_next_instruction_name(),
                            engine=ins.engine,
                            ins=[],
                            outs=[],
                            sync_info=mybir.SyncInfo(on_wait=[w], on_update=[]),
                        )
                        nc.register_instruction(nop, overwrite=True)
                        out.append(nop)
                    si.on_wait = waits[:1]
                    changed = True
                out.append(ins)
            if changed:
                bb.instructions[:] = out


# ---- ntff hook shim (enables trace=True under this image) ----
def _install_ntff_hook():
    import types, contextlib, ctypes

    if "antenv.axon_hooks" in sys.modules:
        return
    try:
        from trn_agent_boot.trn_boot import _ntff_profile_via_ctypes

        hook = _ntff_profile_via_ctypes("/opt/axon/libaxon_pjrt.so")
    except Exception:
        hook = None
    mod = types.ModuleType("antenv.axon_hooks")
    mod.get_axon_ntff_profile_hook = lambda: hook
    mod.set_axon_ntff_profile_hook = lambda h: None
    sys.modules["antenv.axon_hooks"] = mod
    import antenv

    antenv.axon_hooks = mod


NC = 8
P = 128
DP = 128          # padded feature width (bytes row = 256)
WT = 256          # dst-tile width
WIN = 32768       # int16 gather window
CALL = 1024       # rows per dma_gather call
f32 = mybir.dt.float32
bf16 = mybir.dt.bfloat16
i16 = mybir.dt.int16

H = 120
NU, NT, NL = 100000, 100000, 10000
SH = {"user": NU // NC, "tweet": NT // NC, "list": NL // NC}
NFULL = {"user": NU, "tweet": NT, "list": NL}
DF = {"user": 120, "tweet": 90, "list": 60}   # real feature dims

# relations per destination type: (edge_name, src_type)
RELS = {
    "user": [
        ("uu_following", "user"), ("uu_rev_following", "user"),
        ("tu_post", "tweet"), ("tu_like", "tweet"), ("tu_pin", "tweet"),
        ("lu_member", "list"), ("lu_following", "list"), ("lu_own", "list"),
    ],
    "tweet": [("ut_rev_post", "user"), ("ut_rev_like", "user"),
              ("ut_rev_pin", "user")],
    "list": [("ul_rev_following", "user"), ("ul_rev_member", "user"),
             ("ul_rev_own", "user")],
}


def _pad2(a, r, c):
    out = np.zeros((r, c), np.float32)
    out[: a.shape[0], : a.shape[1]] = a
    return out


def _padv(a, r):
    out = np.zeros((r,), np.float32)
    out[: a.shape[0]] = a
    return out


def preprocess(inputs):
    """Shard + build all per-core input tensors and the compile-time dims."""
    p = {k: np.asarray(v) for k, v in inputs["params"].items()}
    cores = [dict() for _ in range(NC)]
    meta = {}

    # ---------- encoder inputs: transposed bf16 shards ----------
    def shard_T(name, arr, sh):
        arr = np.asarray(arr)
        for c in range(NC):
            cores[c][name] = np.ascontiguousarray(
                arr[c * sh : (c + 1) * sh].T.astype(bfloat16)
            )

    shard_T("e_user_desc", inputs["user_desc"], SH["user"])
    shard_T("e_user_num", inputs["user_num"], SH["user"])
    shard_T("e_user_cat", inputs["user_cat"], SH["user"])
    shard_T("e_tweet_num", inputs["tweet_num"], SH["tweet"])
    shard_T("e_tweet_text", inputs["tweet_text"], SH["tweet"])
    shard_T("e_list_num", inputs["list_num"], SH["list"])
    shard_T("e_list_desc", inputs["list_desc"], SH["list"])

    # ---------- weights (replicated, padded, bf16 lhsT layout) ----------
    W = {}

    def addW(name, a, kchunk=False):
        a = np.asarray(a, np.float32)
        if kchunk:  # [768, M] -> [128, 6*M]
            k, m = a.shape
            nk = k // P
            W[name] = np.ascontiguousarray(
                a.reshape(nk, P, m).transpose(1, 0, 2).reshape(P, nk * m)
            ).astype(bfloat16)
        else:
            r = P if a.shape[0] > 1 else a.shape[0]
            W[name] = _pad2(a, r, max(a.shape[1], 1)).astype(bfloat16)

    def addB(name, a, r=P):
        W[name] = _padv(np.asarray(a, np.float32), r)[:, None].astype(np.float32)

    addW("Wud", p["l_user_desc_W"], kchunk=True); addB("bud", p["l_user_desc_b"], 40)
    addW("Wun", p["l_user_num_W"]); addB("bun", p["l_user_num_b"], 40)
    addW("Wuc", p["l_user_cat_W"]); addB("buc", p["l_user_cat_b"], 40)
    Wui = np.asarray(p["l_user_input_W"], np.float32)
    W["Wui_0"] = _pad2(Wui[0:40], 40, P).astype(bfloat16)
    W["Wui_1"] = _pad2(Wui[40:80], 40, P).astype(bfloat16)
    W["Wui_2"] = _pad2(Wui[80:120], 40, P).astype(bfloat16)
    addB("bui", p["l_user_input_b"])
    addW("Wtn", p["l_tweet_num_W"]); addB("btn", p["l_tweet_num_b"], 30)
    addW("Wtt", p["l_tweet_text_W"], kchunk=True); addB("btt", p["l_tweet_text_b"], 60)
    Wti = np.asarray(p["l_tweet_input_W"], np.float32)
    W["Wti_0"] = _pad2(Wti[0:30], 30, P).astype(bfloat16)
    W["Wti_1"] = _pad2(Wti[30:90], 60, P).astype(bfloat16)
    addB("bti", p["l_tweet_input_b"])
    addW("Wln", p["l_list_num_W"]); addB("bln", p["l_list_num_b"], 30)
    addW("Wld", p["l_list_desc_W"], kchunk=True); addB("bld", p["l_list_desc_b"], 30)
    Wli = np.asarray(p["l_list_input_W"], np.float32)
    W["Wli_0"] = _pad2(Wli[0:30], 30, P).astype(bfloat16)
    W["Wli_1"] = _pad2(Wli[30:60], 30, P).astype(bfloat16)
    addB("bli", p["l_list_input_b"])

    def permrows(a, d):
        h2 = DF[d] // 2
        out = np.zeros((P, a.shape[1]), np.float32)
        out[0:h2] = a[0:h2]
        out[64 : 64 + h2] = a[h2 : 2 * h2]
        return out

    # sage weights per phase
    for L in ("1", "2"):
        srcty = ["user", "user", "tweet", "tweet", "tweet", "list", "list", "list"]
        for i in range(8):
            a = np.asarray(p[f"user_sage{L}{i+1}_rel_W"], np.float32)
            if L == "2":
                a = permrows(a, srcty[i])
            W[f"Wrel_u{L}_{i}"] = _pad2(a, P, P).astype(bfloat16)
        addB(f"bneigh_u{L}", sum(p[f"user_sage{L}{i+1}_rel_b"] for i in range(8)))
        rt = np.asarray(sum(p[f"user_sage{L}{i+1}_root_W"] for i in range(8)), np.float32)
        if L == "2":
            rt = permrows(rt, "user")
        W[f"Wroot_u{L}"] = _pad2(rt, P, P).astype(bfloat16)
    for i in range(3):
        W[f"Wrel_t_{i}"] = _pad2(p[f"tweet_sage1{i+1}_rel_W"], P, P).astype(bfloat16)
        W[f"Wrel_l_{i}"] = _pad2(p[f"list_sage1{i+1}_rel_W"], P, P).astype(bfloat16)
    addB("bneigh_t", sum(p[f"tweet_sage1{i+1}_rel_b"] for i in range(3)))
    addB("bneigh_l", sum(p[f"list_sage1{i+1}_rel_b"] for i in range(3)))
    W["Wroot_t"] = _pad2(sum(p[f"tweet_sage1{i+1}_root_W"] for i in range(3)), P, P).astype(bfloat16)
    W["Wroot_l"] = _pad2(sum(p[f"list_sage1{i+1}_root_W"] for i in range(3)), P, P).astype(bfloat16)

    W["Wself_u"] = _pad2(p["user_self1_W"], P, 64).astype(bfloat16)
    W["Wself_u_p"] = _pad2(permrows(np.asarray(p["user_self1_W"], np.float32), "user"), P, 64).astype(bfloat16)
    addB("bself_u", p["user_self1_b"], 64)
    W["Wnei_u"] = _pad2(p["user_neigh1_W"], P, 64).astype(bfloat16)
    addB("bnei_u", p["user_neigh1_b"], 64)
    W["Wself_t"] = _pad2(p["tweet_self1_W"], P, 64).astype(bfloat16)
    addB("bself_t", p["tweet_self1_b"], 64)
    W["Wnei_t"] = _pad2(p["tweet_neigh1_W"], P, 64).astype(bfloat16)
    addB("bnei_t", p["tweet_neigh1_b"], 64)
    W["Wself_l"] = _pad2(p["list_self1_W"], P, 64).astype(bfloat16)
    addB("bself_l", p["list_self1_b"], 64)
    W["Wnei_l"] = _pad2(p["list_neigh1_W"], P, 64).astype(bfloat16)
    addB("bnei_l", p["list_neigh1_b"], 64)

    W["Wout"] = _pad2(permrows(np.asarray(p["l_user_output_W"], np.float32), "user"), P, P).astype(bfloat16)
    addB("bout", p["l_user_output_b"])
    W["Wpred"] = _pad2(p["l_prediction_W"], P, 2).astype(bfloat16)
    addB("bpred", p["l_prediction_b"], 2)

    W["iota"] = np.broadcast_to(
        np.arange(WT, dtype=np.float32).astype(bfloat16), (P, WT)
    ).copy()
    W["id128"] = np.eye(P, dtype=np.float32).astype(bfloat16)
    W["ones"] = np.ones((P, 1), np.float32).astype(bfloat16)
    W["ones1"] = np.ones((1, P), np.float32)
    for c in range(NC):
        cores[c].update(W)

    # ---------- edge metadata ----------
    # per dst-type d: streams per (r, w): t-major, per-(t,w) padded to x128,
    # chunk counts = max over cores
    for d in ("user", "tweet", "list"):
        sh = SH[d]
        ntile = (sh + WT - 1) // WT
        meta[d] = {"ntile": ntile, "streams": []}
        for ri, (ename, stype) in enumerate(RELS[d]):
            ei = np.asarray(inputs["ei_" + ename])
            ew = np.asarray(inputs["ew_" + ename], np.float32)
            src, dst = ei[0].astype(np.int64), ei[1].astype(np.int64)
            dcore = dst // sh
            local = dst % sh
            tl = local // WT
            drel = (local % WT).astype(np.float32)
            win = src // WIN
            nw = int(np.ceil(NFULL[stype] / WIN))
            for w in range(nw):
                # per-core group sizes
                percore = []
                for c in range(NC):
                    m = (dcore == c) & (win == w)
                    order = np.argsort(tl[m], kind="stable")
                    percore.append(
                        (tl[m][order], (src[m] - w * WIN)[order], drel[m][order], ew[m][order])
                    )
                cnt = np.zeros((NC, ntile), np.int64)
                for c in range(NC):
                    t_arr = percore[c][0]
                    np.add.at(cnt[c], t_arr, 1)
                mx = cnt.max(0)
                chunks = (mx + 127) // 128          # padded chunks per tile
                choff = np.concatenate([[0], np.cumsum(chunks)])
                C = int(choff[-1])                  # total chunks in stream
                R = C * 128
                idx_all, dst_all, ew_all = [], [], []
                for c in range(NC):
                    t_arr, s_arr, dr_arr, w_arr = percore[c]
                    sid = np.zeros(R, np.int16)
                    sdr = np.full(R, -1.0, np.float32)
                    sew = np.zeros(R, np.float32)
                    # place each tile's edges at its padded offset
                    starts = choff[:-1] * 128
                    pos = starts[t_arr] + (
                        np.arange(len(t_arr))
                        - np.concatenate([[0], np.cumsum(cnt[c])])[t_arr]
                    )
                    sid[pos] = s_arr.astype(np.int16)
                    sdr[pos] = dr_arr
                    sew[pos] = w_arr
                    idx_all.append(sid)
                    dst_all.append(sdr)
                    ew_all.append(sew)
                # tensors: idx wrapped [128, R/16]; meta [128, C]
                C8 = ((C + 7) // 8) * 8
                nm_i = f"gi_{d}_{ri}_{w}"
                nm_d = f"gd_{d}_{ri}_{w}"
                nm_e = f"ge_{d}_{ri}_{w}"
                for c in range(NC):
                    idxp = np.zeros(C8 * 128, np.int16)
                    idxp[:R] = idx_all[c]
                    wrap = np.ascontiguousarray(idxp.reshape(-1, 16).T)
                    cores[c][nm_i] = np.tile(wrap, (8, 1))
                    cores[c][nm_d] = np.ascontiguousarray(
                        dst_all[c].reshape(C, 128).T
                    )
                    cores[c][nm_e] = np.ascontiguousarray(
                        ew_all[c].reshape(C, 128).T
                    )
                meta[d]["streams"].append(
                    dict(ri=ri, w=w, stype=stype, chunks=chunks, choff=choff,
                         C=C, C8=C8)
                )
    return cores, meta


def build(meta):
    nc = bacc.Bacc("TRN2", target_bir_lowering=False, num_devices=NC,
                   num_swdge_queues=4)
    AF = mybir.ActivationFunctionType
    OP = mybir.AluOpType

    inp = {}

    def dram(name, shape, dt):
        inp[name] = nc.dram_tensor(name, list(shape), dt, kind="ExternalInput")
        return inp[name]

    # encoder inputs
    dram("e_user_desc", (768, SH["user"]), bf16)
    dram("e_user_num", (11, SH["user"]), bf16)
    dram("e_user_cat", (7, SH["user"]), bf16)
    dram("e_tweet_num", (14, SH["tweet"]), bf16)
    dram("e_tweet_text", (768, SH["tweet"]), bf16)
    dram("e_list_num", (5, SH["list"]), bf16)
    dram("e_list_desc", (768, SH["list"]), bf16)
    # weights
    wnames = {}
    WSHAPES = dict(
        Wud=(P, 240), Wun=(11, 40), Wuc=(7, 40),
        Wui_0=(40, P), Wui_1=(40, P), Wui_2=(40, P),
        Wtn=(14, 30), Wtt=(P, 360), Wti_0=(30, P), Wti_1=(60, P),
        Wln=(5, 30), Wld=(P, 180), Wli_0=(30, P), Wli_1=(30, P),
        Wout=(P, P), Wpred=(P, 2), iota=(P, WT), id128=(P, P),
        ones=(P, 1),
        Wself_u=(P, 64), Wself_u_p=(P, 64), Wnei_u=(P, 64),
        Wself_t=(P, 64), Wnei_t=(P, 64),
        Wself_l=(P, 64), Wnei_l=(P, 64),
        Wroot_u1=(P, P), Wroot_u2=(P, P), Wroot_t=(P, P), Wroot_l=(P, P),
    )
    for L in ("1", "2"):
        for i in range(8):
            WSHAPES[f"Wrel_u{L}_{i}"] = (P, P)
    for i in range(3):
        WSHAPES[f"Wrel_t_{i}"] = (P, P)
        WSHAPES[f"Wrel_l_{i}"] = (P, P)
    BSHAPES = dict(
        bud=(40, 1), bun=(40, 1), buc=(40, 1), bui=(P, 1), btn=(30, 1),
        btt=(60, 1), bti=(P, 1), bln=(30, 1), bld=(30, 1), bli=(P, 1),
        bneigh_u1=(P, 1), bneigh_u2=(P, 1), bneigh_t=(P, 1), bneigh_l=(P, 1),
        bself_u=(64, 1), bnei_u=(64, 1), bself_t=(64, 1), bnei_t=(64, 1),
        bself_l=(64, 1), bnei_l=(64, 1), bout=(P, 1), bpred=(2, 1),
        ones1=(1, P),
    )
    for n, s in WSHAPES.items():
        dram(n, s, bf16)
    for n, s in BSHAPES.items():
        dram(n, s, f32)
    # edge metadata tensors
    for d in ("user", "tweet", "list"):
        for st in meta[d]["streams"]:
            C = st["C"]
            dram(f"gi_{d}_{st['ri']}_{st['w']}", (P, st["C8"] * 8), i16)
            dram(f"gd_{d}_{st['ri']}_{st['w']}", (P, C), f32)
            dram(f"ge_{d}_{st['ri']}_{st['w']}", (P, C), f32)

    out_T = nc.dram_tensor("out_T", [2, SH["user"]], f32, kind="ExternalOutput")

    with tile.TileContext(nc) as tc:
        with (
            tc.tile_pool(name="wpool", bufs=1) as wp,
            tc.tile_pool(name="sb", bufs=3) as sb,
            tc.tile_pool(name="fpool", bufs=2) as fp,
            tc.tile_pool(name="spool", bufs=4) as sp,
            tc.tile_pool(name="ps", bufs=1, space="PSUM") as ps,
            tc.tile_pool(name="ps2", bufs=4, space="PSUM") as ps2,
            tc.tile_pool(name="dram", bufs=1, space="DRAM") as dp,
        ):
            nc.gpsimd.load_library(library_config.mlp)
            nreg = nc.gpsimd.alloc_register("nidx")
            nc.gpsimd.reg_mov(nreg, CALL)

            # resident weights
            wsb = {}
            for n, s in WSHAPES.items():
                wsb[n] = wp.tile(list(s), bf16, tag=f"w_{n}", name=f"w_{n}")
                nc.sync.dma_start(wsb[n][:], inp[n][:, :])
            for n, s in BSHAPES.items():
                wsb[n] = wp.tile(list(s), f32, tag=f"w_{n}", name=f"w_{n}")
                nc.sync.dma_start(wsb[n][:], inp[n][:, :])

            # feature tables (DRAM): local shard + allgathered full, per layer
            tbl_loc = {}
            tbl_full = {}
            for lay in (0, 1):
                for d in ("user", "tweet", "list"):
                    if lay == 1 and d == "user":
                        pass
                    tbl_loc[(lay, d)] = dp.tile([SH[d], DP], bf16, name=f"tl_{lay}_{d}", tag=f"tl_{lay}_{d}")
                    tbl_full[(lay, d)] = dp.tile([NFULL[d], DP], bf16, name=f"tf_{lay}_{d}", tag=f"tf_{lay}_{d}")

            # ---------- helpers ----------
            def norm_rows_write(xsb, n0, nrows, col0, inv, table, roff):
                """transpose xsb[:, col0:col0+nrows] (feat-major chunk of <=128
                nodes), scale rows by inv [128,1], write bf16 rows to table."""
                tp = ps2.tile([P, P], bf16, tag="scr", space="PSUM")
                nc.tensor.transpose(
                    out=tp[:, :P], in_=xsb[:, col0 : col0 + P],
                    identity=wsb["id128"][:],
                )
                rows = sb.tile([P, P], bf16, tag="rows")
                nc.vector.tensor_scalar(
                    out=rows[:], in0=tp[:], scalar1=inv[:, :1], scalar2=None,
                    op0=OP.mult,
                )
                nc.sync.dma_start(table[roff : roff + nrows, :], rows[:nrows, :])

            def col_norm_inv(sq, col0, ncols):
                """per-node inv norm for nodes col0..col0+127 (feat-major sq)."""
                ssq = ps2.tile([P, 1], f32, tag="scr", space="PSUM")
                nc.tensor.matmul(
                    out=ssq[:], lhsT=sq[:, col0 : col0 + P], rhs=wsb["ones"][:],
                    start=True, stop=True,
                )
                sn = sb.tile([P, 1], f32, tag="sn")
                nc.scalar.activation(out=sn[:], in_=ssq[:], func=AF.Sqrt)
                nc.vector.tensor_scalar(
                    out=sn[:], in0=sn[:], scalar1=1e-12, scalar2=None, op0=OP.max
                )
                inv = sb.tile([P, 1], f32, tag="inv")
                nc.vector.reciprocal(inv[:], sn[:])
                return inv

            # ---------- encoders ----------
            def encoder(d, pieces, bin_name, table):
                sh = SH[d]
                for i0 in range(0, sh, 512):
                    n = min(512, sh - i0)
                    x2p = ps.tile([P, 512], f32, tag="x2p", space="PSUM")
                    np_ = len(pieces)
                    for pi, (src_nm, w_nm, b_nm, wi_nm, m, kchunks, kdim) in enumerate(pieces):
                        pp = ps.tile([m, 512], f32, tag="enc", space="PSUM")
                        if kchunks > 1:
                            for k in range(kchunks):
                                rin = sb.tile([P, 512], bf16, tag="encin")
                                nc.sync.dma_start(
                                    rin[:, :n],
                                    inp[src_nm][k * P : (k + 1) * P, i0 : i0 + n],
                                )
                                nc.tensor.matmul(
                                    out=pp[:, :n],
                                    lhsT=wsb[w_nm][:, k * m : (k + 1) * m],
                                    rhs=rin[:, :n],
                                    start=(k == 0), stop=(k == kchunks - 1),
                                )
                        else:
                            rin = sb.tile([kdim, 512], bf16, tag=f"encin_{kdim}")
                            nc.sync.dma_start(rin[:, :n], inp[src_nm][:, i0 : i0 + n])
                            nc.tensor.matmul(
                                out=pp[:, :n], lhsT=wsb[w_nm][:], rhs=rin[:, :n],
                                start=True, stop=True,
                            )
                        psb = sb.tile([m, 512], bf16, tag=f"psb_{m}")
                        nc.scalar.activation(
                            out=psb[:, :n], in_=pp[:, :n], func=AF.Lrelu,
                            bias=wsb[b_nm][:, :1], alpha=0.01,
                        )
                        nc.tensor.matmul(
                            out=x2p[:, :n], lhsT=wsb[wi_nm][:], rhs=psb[:, :n],
                            start=(pi == 0), stop=(pi == np_ - 1),
                        )
                    x2 = sb.tile([P, 512], bf16, tag="x2")
                    nc.scalar.activation(
                        out=x2[:, :n], in_=x2p[:, :n], func=AF.Lrelu,
                        bias=wsb[bin_name][:, :1], alpha=0.01,
                    )
                    sq = sb.tile([P, 512], bf16, tag="sqe")
                    nc.scalar.activation(out=sq[:, :n], in_=x2[:, :n], func=AF.Square)
                    for c0 in range(0, n, P):
                        nn = min(P, n - c0)
                        if nn < P:
                            nc.vector.memset(x2[:, c0 + nn : c0 + P], 0.0)
                            nc.vector.memset(sq[:, c0 + nn : c0 + P], 0.0)
                        inv = col_norm_inv(sq, c0, nn)
                        norm_rows_write(x2, 0, nn, c0, inv, table, i0 + c0)

            encoder(
                "user",
                [
                    ("e_user_desc", "Wud", "bud", "Wui_0", 40, 6, 768),
                    ("e_user_num", "Wun", "bun", "Wui_1", 40, 1, 11),
                    ("e_user_cat", "Wuc", "buc", "Wui_2", 40, 1, 7),
                ],
                "bui", tbl_loc[(0, "user")],
            )
            encoder(
                "tweet",
                [
                    ("e_tweet_num", "Wtn", "btn", "Wti_0", 30, 1, 14),
                    ("e_tweet_text", "Wtt", "btt", "Wti_1", 60, 6, 768),
                ],
                "bti", tbl_loc[(0, "tweet")],
            )
            encoder(
                "list",
                [
                    ("e_list_num", "Wln", "bln", "Wli_0", 30, 1, 5),
                    ("e_list_desc", "Wld", "bld", "Wli_1", 30, 6, 768),
                ],
                "bli", tbl_loc[(0, "list")],
            )

            def allgather(lay, d):
                nc.gpsimd.collective_compute(
                    "AllGather", mybir.AluOpType.bypass,
                    replica_groups=[list(range(NC))],
                    ins=[tbl_loc[(lay, d)].opt()],
                    outs=[tbl_full[(lay, d)].opt()],
                )

            for d in ("user", "tweet", "list"):
                allgather(0, d)

            # ---------- aggregation phase ----------
            def agg_phase(lay, d, write_rows):
                """lay: which tables to gather from; d: dst type."""
                sh, ntile = SH[d], meta[d]["ntile"]
                dreal = DF[d]
                streams = meta[d]["streams"]
                nrel = len(RELS[d])
                sfx = {"user": "u", "tweet": "t", "list": "l"}[d]
                relw = (
                    [f"Wrel_u{lay+1}_{i}" for i in range(8)]
                    if d == "user"
                    else [f"Wrel_{sfx}_{i}" for i in range(nrel)]
                )
                rootw = f"Wroot_u{lay+1}" if d == "user" else f"Wroot_{sfx}"
                bneigh = f"bneigh_u{lay+1}" if d == "user" else f"bneigh_{sfx}"
                # per-stream gather state
                gst = {}
                for si, st in enumerate(streams):
                    gst[si] = dict(next_chunk=0, bufs={}, st=st)

                def ensure_gathered(si, ci):
                    g = gst[si]
                    st = g["st"]
                    callk = ci // 8
                    if callk in g["bufs"]:
                        return g["bufs"][callk]
                    c0 = callk * 8
                    tblsrc = tbl_full[(lay, st["stype"])]
                    w0 = st["w"] * WIN
                    wlen = min(WIN, NFULL[st["stype"]] - w0)
                    idxt = sb.tile([P, 64], i16, tag="gidx")
                    nc.sync.dma_start(
                        idxt[:],
                        inp[f"gi_{d}_{st['ri']}_{st['w']}"][
                            :, c0 * 8 : c0 * 8 + 64
                        ],
                    )
                    fb = fp.tile([P, 8 * DP], bf16, tag=f"F_{d}_{si}")
                    nc.gpsimd.dma_gather(
                        out_ap=fb[:].rearrange("p (c d) -> p c d", d=DP),
                        in_ap=tblsrc[w0 : w0 + wlen, :],
                        idxs_ap=idxt[:],
                        num_idxs=CALL,
                        num_idxs_reg=nreg,
                        elem_size=DP,
                        single_packet=False,
                        queue_num=callk % 4,
                    )
                    g["bufs"] = {callk: fb}  # keep only latest (consumed in order)
                    return fb

                for t in range(ntile):
                    nd = min(WT, sh - t * WT)
                    neigh = ps.tile([P, WT], f32, tag="neigh", space="PSUM")
                    first_mm2 = True
                    for r in range(nrel):
                        work = []
                        for si, st in enumerate(streams):
                            if st["ri"] != r:
                                continue
                            nch = int(st["chunks"][t])
                            if nch == 0:
                                continue
                            work.append((si, st, int(st["choff"][t]), nch))
                        if not work:
                            continue
                        tot = sum(w[3] for w in work)
                        aggp = ps.tile([P, WT], f32, tag="agg", space="PSUM")
                        nmm = 0
                        for (si, st, co, nch) in work:
                            mdt = sb.tile([P, nch], f32, tag="md")
                            met = sb.tile([P, nch], f32, tag="me")
                            nc.sync.dma_start(
                                mdt[:],
                                inp[f"gd_{d}_{st['ri']}_{st['w']}"][:, co : co + nch],
                            )
                            nc.sync.dma_start(
                                met[:],
                                inp[f"ge_{d}_{st['ri']}_{st['w']}"][:, co : co + nch],
                            )
                            for j in range(nch):
                                ci = co + j
                                fb = ensure_gathered(si, ci)
                                slot = ci % 8
                                S = sp.tile([P, WT], bf16, tag="S")
                                nc.vector.tensor_scalar(
                                    out=S[:], in0=wsb["iota"][:],
                                    scalar1=mdt[:, j : j + 1],
                                    scalar2=met[:, j : j + 1],
                                    op0=OP.is_equal, op1=OP.mult,
                                )
                                nc.tensor.matmul(
                                    out=aggp[:],
                                    lhsT=fb[:, slot * DP : (slot + 1) * DP],
                                    rhs=S[:],
                                    start=(nmm == 0), stop=(nmm == tot - 1),
                                )
                                nmm += 1
                        aggsb = sb.tile([P, WT], bf16, tag="aggsb")
                        nc.scalar.activation(out=aggsb[:], in_=aggp[:], func=AF.Copy)
                        nc.tensor.matmul(
                            out=neigh[:], lhsT=wsb[relw[r]][:], rhs=aggsb[:],
                            start=first_mm2, stop=False,
                        )
                        first_mm2 = False
                    # root term: local x_dst rows -> transposed
                    xT = sb.tile([P, WT], bf16, tag="xT")
                    for c0 in range(0, WT, P):
                        nn = max(0, min(P, nd - c0))
                        if nn == 0:
                            nc.vector.memset(xT[:, c0 : c0 + P], 0.0)
                            continue
                        xr = sb.tile([P, DP], bf16, tag="xr")
                        if nn < P:
                            nc.vector.memset(xr[:], 0.0)
                        nc.sync.dma_start(
                            xr[:nn, :],
                            tbl_loc[(lay, d)][t * WT + c0 : t * WT + c0 + nn, :],
                        )
                        tp = ps2.tile([P, P], bf16, tag="scr", space="PSUM")
                        nc.tensor.transpose(
                            out=tp[:], in_=xr[:], identity=wsb["id128"][:]
                        )
                        nc.scalar.activation(
                            out=xT[:, c0 : c0 + P], in_=tp[:], func=AF.Copy
                        )
                    nc.tensor.matmul(
                        out=neigh[:], lhsT=wsb[rootw][:], rhs=xT[:],
                        start=False, stop=True,
                    )
                    neighsb = sb.tile([P, WT], bf16, tag="neighsb")
                    nc.scalar.activation(
                        out=neighsb[:], in_=neigh[:], func=AF.Identity,
                        bias=wsb[bneigh][:, :1],
                    )
                    h2 = dreal // 2
                    selfw = (
                        "Wself_u_p" if (d == "user" and lay == 1) else f"Wself_{sfx}"
                    )
                    ys = sb.tile([64, WT], f32, tag="ys")
                    yn = sb.tile([64, WT], f32, tag="yn")
                    nc.vector.memset(ys[:], 0.0)
                    nc.vector.memset(yn[:], 0.0)
                    sfp = ps2.tile([64, WT], f32, tag="scr", space="PSUM")
                    nc.tensor.matmul(
                        out=sfp[:], lhsT=wsb[selfw][:], rhs=xT[:],
                        start=True, stop=True,
                    )
                    nc.scalar.activation(
                        out=ys[0:h2, :], in_=sfp[:h2, :], func=AF.Lrelu,
                        bias=wsb[f"bself_{sfx}"][:h2, :1], alpha=0.01,
                    )
                    nfp = ps2.tile([64, WT], f32, tag="scr", space="PSUM")
                    nc.tensor.matmul(
                        out=nfp[:], lhsT=wsb[f"Wnei_{sfx}"][:], rhs=neighsb[:],
                        start=True, stop=True,
                    )
                    nc.scalar.activation(
                        out=yn[0:h2, :], in_=nfp[:h2, :], func=AF.Lrelu,
                        bias=wsb[f"bnei_{sfx}"][:h2, :1], alpha=0.01,
                    )
                    sqs = sb.tile([64, WT], bf16, tag="sqs")
                    sqn = sb.tile([64, WT], bf16, tag="sqn")
                    nc.scalar.activation(out=sqs[:], in_=ys[:], func=AF.Square)
                    nc.scalar.activation(out=sqn[:], in_=yn[:], func=AF.Square)
                    ysb = sb.tile([64, WT], bf16, tag="ysb")
                    ynb = sb.tile([64, WT], bf16, tag="ynb")
                    nc.vector.tensor_copy(ysb[:], ys[:])
                    nc.vector.tensor_copy(ynb[:], yn[:])

                    def colinv2(c0):
                        ssq = ps2.tile([P, 1], f32, tag="scr", space="PSUM")
                        nc.tensor.matmul(
                            out=ssq[:], lhsT=sqs[:, c0 : c0 + P],
                            rhs=wsb["ones"][:64, :], start=True, stop=False,
                        )
                        nc.tensor.matmul(
                            out=ssq[:], lhsT=sqn[:, c0 : c0 + P],
                            rhs=wsb["ones"][:64, :], start=False, stop=True,
                        )
                        sn = sb.tile([P, 1], f32, tag="sn")
                        nc.scalar.activation(out=sn[:], in_=ssq[:], func=AF.Sqrt)
                        nc.vector.tensor_scalar(
                            out=sn[:], in0=sn[:], scalar1=1e-12, scalar2=None,
                            op0=OP.max,
                        )
                        inv = sb.tile([P, 1], f32, tag="inv")
                        nc.vector.reciprocal(inv[:], sn[:])
                        return inv

                    if write_rows:
                        for c0 in range(0, nd, P):
                            nn = min(P, nd - c0)
                            inv = colinv2(c0)
                            rows = sb.tile([P, P], bf16, tag="rows")
                            for half, src in ((0, ysb), (1, ynb)):
                                tp = ps2.tile([P, 64], bf16, tag="scr", space="PSUM")
                                nc.tensor.transpose(
                                    out=tp[:], in_=src[:, c0 : c0 + P],
                                    identity=wsb["id128"][:64, :64],
                                )
                                nc.vector.tensor_scalar(
                                    out=rows[:, half * 64 : half * 64 + 64],
                                    in0=tp[:], scalar1=inv[:, :1], scalar2=None,
                                    op0=OP.mult,
                                )
                            nc.sync.dma_start(
                                tbl_loc[(1, d)][
                                    t * WT + c0 : t * WT + c0 + nn, :
                                ],
                                rows[:nn, :],
                            )
                    else:
                        # layer2 user: per-column norm then output head
                        ssqr = ps2.tile([1, WT], f32, tag="scr", space="PSUM")
                        nc.tensor.matmul(
                            out=ssqr[:], lhsT=wsb["ones"][:64, :], rhs=sqs[:],
                            start=True, stop=False,
                        )
                        nc.tensor.matmul(
                            out=ssqr[:], lhsT=wsb["ones"][:64, :], rhs=sqn[:],
                            start=False, stop=True,
                        )
                        snr = sb.tile([1, WT], f32, tag="snr")
                        nc.scalar.activation(out=snr[:], in_=ssqr[:], func=AF.Sqrt)
                        nc.vector.tensor_scalar(
                            out=snr[:], in0=snr[:], scalar1=1e-12, scalar2=None,
                            op0=OP.max,
                        )
                        invr = sb.tile([1, WT], f32, tag="invr")
                        nc.vector.reciprocal(invr[:], snr[:])
                        invrep = ps2.tile([P, WT], f32, tag="scr", space="PSUM")
                        nc.tensor.matmul(
                            out=invrep[:], lhsT=wsb["ones1"][:], rhs=invr[:],
                            start=True, stop=True,
                        )
                        ux2 = sb.tile([P, WT], bf16, tag="ux2")
                        nc.vector.tensor_tensor(
                            out=ux2[0:64, :], in0=ys[:], in1=invrep[0:64, :],
                            op=OP.mult,
                        )
                        nc.vector.tensor_tensor(
                            out=ux2[64:128, :], in0=yn[:], in1=invrep[0:64, :],
                            op=OP.mult,
                        )
                        o1p = ps2.tile([P, WT], f32, tag="scr", space="PSUM")
                        nc.tensor.matmul(
                            out=o1p[:], lhsT=wsb["Wout"][:], rhs=ux2[:],
                            start=True, stop=True,
                        )
                        o1 = sb.tile([P, WT], bf16, tag="o1")
                        nc.scalar.activation(
                            out=o1[:], in_=o1p[:], func=AF.Lrelu,
                            bias=wsb["bout"][:, :1], alpha=0.01,
                        )
                        prp = ps2.tile([2, WT], f32, tag="scr", space="PSUM")
                        nc.tensor.matmul(
                            out=prp[:], lhsT=wsb["Wpred"][:], rhs=o1[:],
                            start=True, stop=True,
                        )
                        pr = sb.tile([2, WT], f32, tag="pr")
                        nc.scalar.activation(
                            out=pr[:], in_=prp[:], func=AF.Identity,
                            bias=wsb["bpred"][:, :1],
                        )
                        nc.sync.dma_start(
                            out_T[:, t * WT : t * WT + nd], pr[:, :nd]
                        )

            agg_phase(0, "tweet", True)
            agg_phase(0, "list", True)
            agg_phase(0, "user", True)
            for d in ("user", "tweet", "list"):
                allgather(1, d)
            agg_phase(1, "user", False)

    legalize_waits(nc)
    nc.compile()
    return nc


_CACHE = {}


def kernel(**inputs):
    _install_ntff_hook()
    cores, meta = preprocess(inputs)
    key = "k"
    if key not in _CACHE:
        _CACHE[key] = build(meta)
    nc = _CACHE[key]
    res = run_bass_kernel_spmd(nc, cores, core_ids=list(range(NC)))
    out = np.concatenate(
        [np.asarray(res.results[c]["out_T"]).T for c in range(NC)], axis=0
    )
    return out.astype(np.float32)


def run_traced(inputs):
    _install_ntff_hook()
    cores, meta = preprocess(inputs)
    nc = build(meta)
    res = run_bass_kernel_spmd(nc, cores, core_ids=list(range(NC)), trace=True)
    out = np.concatenate(
        [np.asarray(res.results[c]["out_T"]).T for c in range(NC)], axis=0
    )
    return out.astype(np.float32), res
